# revision 11
# baseline (speedup 1.0000x reference)
"""GalaxyTileDecoder on 8 Trainium2 NeuronCores.

The reference pipeline (linear decode -> zero-pad -> gate -> bilinear
grid_sample -> sum over M=2 sources) collapses algebraically: the sample
grid is a pure per-source translation, sampling the padded 53x53 image at
(y, x) = (i + 2.5 - 4*locs[...,0], j + 2.5 - 4*locs[...,1]).  Folding the
integer shift (one-hot over 6 positions per axis), the bilinear weights,
the decoder bias, the galaxy_bool gate, and the M-source sum into an
expanded feature dimension turns the whole forward into one matmul:

    out[p, :] = (sum_par z_exp[p, par, :]) @ W_exp        (K=324)

with W_exp[(a, b, f), (i, j)] = canvas9[f, a+i, b+j] the 6x6 shifted
52x52 windows of the 9 basis images (8 decoder rows + bias) in a 57x57
zero canvas, and z_exp the per-source sparse coefficients
bool * z9[f] * wy[a] * wx[b].  The host computes the tiny coefficient
expansion (~0.002% of FLOPs); the device does the 10000x324x2704 matmul.

Data parallel over the ptile axis: 1250 ptiles per core, no collectives.
"""

import math
import os

import numpy as np

P_TOTAL = 10000
M = 2
N_CORES = 8
PT = P_TOTAL // N_CORES          # ptiles per core
F = 9                            # 8 decoder features + bias
A = 6                            # y-shift positions (-2..3)
B = 6                            # x-shift positions (-2..3)
K = A * B * F                    # 324 expanded features
OUT_HW = 52
COLS = OUT_HW * OUT_HW           # 2704
HALF = COLS // 2                 # 1352
CANVAS = 57

_DT_NAME = os.environ.get("BASS_GAL_DT", "bf16")

_cache = {}


def _build_program(dt_name):
    import concourse.bass as bass  # noqa: F401  (registers engines)
    import concourse.tile as tile
    from concourse import bacc, mybir

    dt_map = {
        "bf16": mybir.dt.bfloat16,
        "f32": mybir.dt.float32,
        "f32r": mybir.dt.float32r,
    }
    DT = dt_map[dt_name]

    n_batches = math.ceil(PT / 128)
    nc = bacc.Bacc(trn_type="TRN2")
    # host-blocked layouts so every DMA reads a fully contiguous DRAM block
    zt = nc.dram_tensor("zt", [K, n_batches * 128], DT, kind="ExternalInput")
    wx = nc.dram_tensor("wx", [K, COLS], DT, kind="ExternalInput")
    out = nc.dram_tensor("out", [PT, COLS], mybir.dt.float32, kind="ExternalOutput")

    KCH = [(0, 128), (128, 256), (256, K)]
    # output split into 2-bank PSUM pieces; segs within a piece are <=512
    PIECES = [(0, 1024), (1024, 2048), (2048, COLS)]
    SEGS = {0: [(0, 512), (512, 1024)],
            1: [(1024, 1536), (1536, 2048)],
            2: [(2048, 2560), (2560, COLS)]}

    with tile.TileContext(nc) as tc:
        with (
            tc.tile_pool(name="w", bufs=1) as wpool,
            tc.tile_pool(name="o", bufs=4) as opool,
            tc.tile_pool(name="ps", bufs=4, space="PSUM") as pspool,
        ):
            # All inputs preloaded upfront on the HWDGE (sync) queue, in the
            # order the first batch consumes them, so the first matmul can
            # start as soon as (w piece0, z) land.
            w_tiles = {}
            z_full = []
            for ci, (k0, k1) in enumerate(KCH):
                p0, p1 = PIECES[0]
                wt = wpool.tile([k1 - k0, p1 - p0], DT, tag=f"w{ci}_0")
                nc.sync.dma_start(wt[:], wx[k0:k1, p0:p1])
                w_tiles[ci, 0] = wt
                zb = wpool.tile([k1 - k0, n_batches * 128], DT, tag=f"z{ci}")
                nc.sync.dma_start(zb[:], zt[k0:k1, :])
                z_full.append(zb)
            for pi in range(1, len(PIECES)):
                p0, p1 = PIECES[pi]
                for ci, (k0, k1) in enumerate(KCH):
                    wt = wpool.tile([k1 - k0, p1 - p0], DT, tag=f"w{ci}_{pi}")
                    nc.sync.dma_start(wt[:], wx[k0:k1, p0:p1])
                    w_tiles[ci, pi] = wt

            for bi in range(n_batches):
                b0 = bi * 128
                bs = min(128, PT - b0)
                z_b = [z_full[ci][:, bi * 128:bi * 128 + bs] for ci in range(3)]
                for pi, (p0, p1) in enumerate(PIECES):
                    pw = p1 - p0
                    ps = pspool.tile([128, 1024], mybir.dt.float32, tag="ps")
                    for ci in range(len(KCH)):
                        for (s0, s1) in SEGS[pi]:
                            nc.tensor.matmul(
                                ps[0:bs, s0 - p0:s1 - p0],
                                z_b[ci][:, 0:bs],
                                w_tiles[ci, pi][:, s0 - p0:s1 - p0],
                                start=(ci == 0),
                                stop=(ci == len(KCH) - 1),
                            )
                    osb = opool.tile([128, 1024], mybir.dt.float32, tag="osb")
                    nc.vector.tensor_copy(osb[0:bs, 0:pw], ps[0:bs, 0:pw])
                    nc.sync.dma_start(out[b0:b0 + bs, p0:p1], osb[0:bs, 0:pw])
    nc.compile()
    return nc


def _get_program(dt_name):
    if dt_name not in _cache:
        _cache[dt_name] = _build_program(dt_name)
    return _cache[dt_name]


def _host_expand(locs, galaxy_params, galaxy_bool, W_dec, b_dec, np_dtype):
    """Build zt (K, P_TOTAL) parity-summed coefficients and Wexp (K, COLS)."""
    locs = np.asarray(locs, np.float32).reshape(-1, 2)
    params = np.asarray(galaxy_params, np.float32).reshape(-1, 8)
    gbool = np.asarray(galaxy_bool, np.float32).reshape(-1, 1)
    W = np.asarray(W_dec, np.float32)
    b = np.asarray(b_dec, np.float32)
    N = locs.shape[0]

    sy = 2.5 - 4.0 * locs[:, 0]
    sx = 2.5 - 4.0 * locs[:, 1]
    m = np.floor(sy)
    k = np.floor(sx)
    fy = (sy - m).astype(np.float32)
    fx = (sx - k).astype(np.float32)
    m = m.astype(np.int64)
    k = k.astype(np.int64)
    ar = np.arange(N)
    cy = np.zeros((N, A), np.float32)
    cx = np.zeros((N, B), np.float32)
    cy[ar, m + 2] = 1.0 - fy
    cy[ar, m + 3] = fy
    cx[ar, k + 2] = 1.0 - fx
    cx[ar, k + 3] = fx

    z9 = np.concatenate([params, np.ones((N, 1), np.float32)], axis=1) * gbool
    z_exp = (cy[:, :, None, None] * cx[:, None, :, None] * z9[:, None, None, :])
    # sum the M=2 sources of each ptile (matmul is linear in z_exp)
    z_sum = z_exp.reshape(P_TOTAL, M, K).sum(axis=1)
    # per core (K, n_batches*128), zero-padded past PT
    n_batches = math.ceil(PT / 128)
    z_blk = np.zeros((N_CORES, K, n_batches * 128), np_dtype)
    zc = z_sum.astype(np_dtype).T.reshape(K, N_CORES, PT)     # (K, core, pt)
    for c in range(N_CORES):
        z_blk[c, :, 0:PT] = zc[:, c, :]

    canvas9 = np.zeros((F, CANVAS, CANVAS), np.float32)
    canvas9[:8, 3:54, 3:54] = W.reshape(8, 51, 51)
    canvas9[8, 3:54, 3:54] = b.reshape(51, 51)
    sw = np.lib.stride_tricks.sliding_window_view(canvas9, (OUT_HW, OUT_HW), axis=(1, 2))
    Wexp = np.ascontiguousarray(
        sw.transpose(1, 2, 0, 3, 4).reshape(K, COLS), dtype=np_dtype)
    return z_blk, Wexp


def kernel(locs, galaxy_params, galaxy_bool, W_dec, b_dec, _trace=False):
    import ml_dtypes
    from concourse.bass_utils import run_bass_kernel_spmd

    np_dtype = {
        "bf16": ml_dtypes.bfloat16,
        "f32": np.float32,
        "f32r": np.float32,
    }[_DT_NAME]

    z_blk, Wexp = _host_expand(
        locs, galaxy_params, galaxy_bool, W_dec, b_dec, np_dtype)

    nc = _get_program(_DT_NAME)
    in_maps = [
        {
            "zt": z_blk[c],
            "wx": Wexp,
        }
        for c in range(N_CORES)
    ]
    kwargs = {}
    if _trace:
        kwargs["trace"] = True
    res = run_bass_kernel_spmd(nc, in_maps, core_ids=list(range(N_CORES)), **kwargs)

    out = np.concatenate([res.results[c]["out"] for c in range(N_CORES)], axis=0)
    out = out.reshape(P_TOTAL, 1, OUT_HW, OUT_HW)
    if _trace:
        kernel._last_result = res
    return out, out


# revision 12
# speedup vs baseline: 1.0103x; 1.0103x over previous
"""GalaxyTileDecoder on 8 Trainium2 NeuronCores.

The reference pipeline (linear decode -> zero-pad -> gate -> bilinear
grid_sample -> sum over M=2 sources) collapses algebraically: the sample
grid is a pure per-source translation, sampling the padded 53x53 image at
(y, x) = (i + 2.5 - 4*locs[...,0], j + 2.5 - 4*locs[...,1]).  Folding the
integer shift (one-hot over 6 positions per axis), the bilinear weights,
the decoder bias, the galaxy_bool gate, and the M-source sum into an
expanded feature dimension turns the whole forward into one matmul:

    out[p, :] = (sum_par z_exp[p, par, :]) @ W_exp        (K=324)

with W_exp[(a, b, f), (i, j)] = canvas9[f, a+i, b+j] the 6x6 shifted
52x52 windows of the 9 basis images (8 decoder rows + bias) in a 57x57
zero canvas, and z_exp the per-source sparse coefficients
bool * z9[f] * wy[a] * wx[b].  The host computes the tiny coefficient
expansion (~0.002% of FLOPs); the device does the 10000x324x2704 matmul.

Data parallel over the ptile axis: 1250 ptiles per core, no collectives.
"""

import math
import os

import numpy as np

P_TOTAL = 10000
M = 2
N_CORES = 8
PT = P_TOTAL // N_CORES          # ptiles per core
F = 9                            # 8 decoder features + bias
A = 6                            # y-shift positions (-2..3)
B = 6                            # x-shift positions (-2..3)
K = A * B * F                    # 324 expanded features
OUT_HW = 52
COLS = OUT_HW * OUT_HW           # 2704
HALF = COLS // 2                 # 1352
CANVAS = 57

_DT_NAME = os.environ.get("BASS_GAL_DT", "bf16")

_cache = {}


def _build_program(dt_name):
    import concourse.bass as bass  # noqa: F401  (registers engines)
    import concourse.tile as tile
    from concourse import bacc, mybir

    dt_map = {
        "bf16": mybir.dt.bfloat16,
        "f32": mybir.dt.float32,
        "f32r": mybir.dt.float32r,
    }
    DT = dt_map[dt_name]

    n_batches = math.ceil(PT / 128)
    nc = bacc.Bacc(trn_type="TRN2")
    # host-blocked layouts so every DMA reads a fully contiguous DRAM block
    zt = nc.dram_tensor("zt", [K, n_batches * 128], DT, kind="ExternalInput")
    wx = nc.dram_tensor("wx", [K, COLS], DT, kind="ExternalInput")
    out = nc.dram_tensor("out", [PT, COLS], mybir.dt.float32, kind="ExternalOutput")

    KCH = [(0, 128), (128, 256), (256, K)]
    # output split into 2-bank PSUM pieces; segs within a piece are <=512
    PIECES = [(0, 1024), (1024, 2048), (2048, COLS)]
    SEGS = {0: [(0, 512), (512, 1024)],
            1: [(1024, 1536), (1536, 2048)],
            2: [(2048, 2560), (2560, COLS)]}

    with tile.TileContext(nc) as tc:
        with (
            tc.tile_pool(name="w", bufs=1) as wpool,
            tc.tile_pool(name="o", bufs=4) as opool,
            tc.tile_pool(name="ps", bufs=3, space="PSUM") as pspool,
            tc.tile_pool(name="wm", bufs=1, space="PSUM") as wmpool,
        ):
            # PE warmup: dummy matmuls spanning the input-load phase so the
            # HAM clock-gate is at 2.4 GHz when the real matmuls start.
            warm = wpool.tile([128, 128], DT, tag="warm")
            nc.vector.memset(warm[:], 0.0)
            wps = wmpool.tile([128, 64], mybir.dt.float32, tag="warmps")
            for _ in range(50):
                nc.tensor.matmul(wps[:, :], warm[:, 0:128], warm[:, 0:64],
                                 start=True, stop=True)
            # All inputs preloaded upfront on the HWDGE (sync) queue, in the
            # order the first batch consumes them, so the first matmul can
            # start as soon as (w piece0, z) land.
            w_tiles = {}
            z_full = []
            for ci, (k0, k1) in enumerate(KCH):
                p0, p1 = PIECES[0]
                wt = wpool.tile([k1 - k0, p1 - p0], DT, tag=f"w{ci}_0")
                nc.sync.dma_start(wt[:], wx[k0:k1, p0:p1])
                w_tiles[ci, 0] = wt
                zb = wpool.tile([k1 - k0, n_batches * 128], DT, tag=f"z{ci}")
                nc.sync.dma_start(zb[:], zt[k0:k1, :])
                z_full.append(zb)
            for pi in range(1, len(PIECES)):
                p0, p1 = PIECES[pi]
                for ci, (k0, k1) in enumerate(KCH):
                    wt = wpool.tile([k1 - k0, p1 - p0], DT, tag=f"w{ci}_{pi}")
                    nc.sync.dma_start(wt[:], wx[k0:k1, p0:p1])
                    w_tiles[ci, pi] = wt

            for bi in range(n_batches):
                b0 = bi * 128
                bs = min(128, PT - b0)
                z_b = [z_full[ci][:, bi * 128:bi * 128 + bs] for ci in range(3)]
                for pi, (p0, p1) in enumerate(PIECES):
                    pw = p1 - p0
                    ps = pspool.tile([128, 1024], mybir.dt.float32, tag="ps")
                    for ci in range(len(KCH)):
                        for (s0, s1) in SEGS[pi]:
                            nc.tensor.matmul(
                                ps[0:bs, s0 - p0:s1 - p0],
                                z_b[ci][:, 0:bs],
                                w_tiles[ci, pi][:, s0 - p0:s1 - p0],
                                start=(ci == 0),
                                stop=(ci == len(KCH) - 1),
                            )
                    osb = opool.tile([128, 1024], mybir.dt.float32, tag="osb")
                    nc.vector.tensor_copy(osb[0:bs, 0:pw], ps[0:bs, 0:pw])
                    nc.sync.dma_start(out[b0:b0 + bs, p0:p1], osb[0:bs, 0:pw])
    nc.compile()
    return nc


def _get_program(dt_name):
    if dt_name not in _cache:
        _cache[dt_name] = _build_program(dt_name)
    return _cache[dt_name]


def _host_expand(locs, galaxy_params, galaxy_bool, W_dec, b_dec, np_dtype):
    """Build zt (K, P_TOTAL) parity-summed coefficients and Wexp (K, COLS)."""
    locs = np.asarray(locs, np.float32).reshape(-1, 2)
    params = np.asarray(galaxy_params, np.float32).reshape(-1, 8)
    gbool = np.asarray(galaxy_bool, np.float32).reshape(-1, 1)
    W = np.asarray(W_dec, np.float32)
    b = np.asarray(b_dec, np.float32)
    N = locs.shape[0]

    sy = 2.5 - 4.0 * locs[:, 0]
    sx = 2.5 - 4.0 * locs[:, 1]
    m = np.floor(sy)
    k = np.floor(sx)
    fy = (sy - m).astype(np.float32)
    fx = (sx - k).astype(np.float32)
    m = m.astype(np.int64)
    k = k.astype(np.int64)
    ar = np.arange(N)
    cy = np.zeros((N, A), np.float32)
    cx = np.zeros((N, B), np.float32)
    cy[ar, m + 2] = 1.0 - fy
    cy[ar, m + 3] = fy
    cx[ar, k + 2] = 1.0 - fx
    cx[ar, k + 3] = fx

    z9 = np.concatenate([params, np.ones((N, 1), np.float32)], axis=1) * gbool
    z_exp = (cy[:, :, None, None] * cx[:, None, :, None] * z9[:, None, None, :])
    # sum the M=2 sources of each ptile (matmul is linear in z_exp)
    z_sum = z_exp.reshape(P_TOTAL, M, K).sum(axis=1)
    # per core (K, n_batches*128), zero-padded past PT
    n_batches = math.ceil(PT / 128)
    z_blk = np.zeros((N_CORES, K, n_batches * 128), np_dtype)
    zc = z_sum.astype(np_dtype).T.reshape(K, N_CORES, PT)     # (K, core, pt)
    for c in range(N_CORES):
        z_blk[c, :, 0:PT] = zc[:, c, :]

    canvas9 = np.zeros((F, CANVAS, CANVAS), np.float32)
    canvas9[:8, 3:54, 3:54] = W.reshape(8, 51, 51)
    canvas9[8, 3:54, 3:54] = b.reshape(51, 51)
    sw = np.lib.stride_tricks.sliding_window_view(canvas9, (OUT_HW, OUT_HW), axis=(1, 2))
    Wexp = np.ascontiguousarray(
        sw.transpose(1, 2, 0, 3, 4).reshape(K, COLS), dtype=np_dtype)
    return z_blk, Wexp


def kernel(locs, galaxy_params, galaxy_bool, W_dec, b_dec, _trace=False):
    import ml_dtypes
    from concourse.bass_utils import run_bass_kernel_spmd

    np_dtype = {
        "bf16": ml_dtypes.bfloat16,
        "f32": np.float32,
        "f32r": np.float32,
    }[_DT_NAME]

    z_blk, Wexp = _host_expand(
        locs, galaxy_params, galaxy_bool, W_dec, b_dec, np_dtype)

    nc = _get_program(_DT_NAME)
    in_maps = [
        {
            "zt": z_blk[c],
            "wx": Wexp,
        }
        for c in range(N_CORES)
    ]
    kwargs = {}
    if _trace:
        kwargs["trace"] = True
    res = run_bass_kernel_spmd(nc, in_maps, core_ids=list(range(N_CORES)), **kwargs)

    out = np.concatenate([res.results[c]["out"] for c in range(N_CORES)], axis=0)
    out = out.reshape(P_TOTAL, 1, OUT_HW, OUT_HW)
    if _trace:
        kernel._last_result = res
    return out, out
